# revision 21
# baseline (speedup 1.0000x reference)
"""Trainium2 Bass kernel for the DeepBSDE loss (nn_BaseDeepBSDE).

Data-parallel over 8 NeuronCores: each core simulates 2048 Monte-Carlo
paths through the 100-step SDE loop and produces a partial loss sum;
the host gathers the 8 partial scalars.

Device-side layout (per core, Bc = 2048 paths):
  - folded state layout: [128 partitions, 16] with path b = c*128 + p
  - two software-pipelined path groups (A: chunks 0-7, B: 8-15) whose
    MLP stages interleave across engines; each group keeps its y state
    in a persistent PSUM tile that the per-step transpose matmul
    accumulates into directly (start=False accumulation)
  - MLP activations feature-major: [128 features (z-MLP 0:63 | q-MLP
    64:127), batch free-dim], bf16 matmuls with fp32 PSUM accumulate
  - relu chunks alternate Scalar/Vector (the only PSUM-capable
    engines); epilogue runs on GpSimd from an SBUF copy of zq
  - noise tensors pre-folded on host to [128, steps*48]; sw-prepass
    folds the y0 initial condition into the step-0 drift term
"""

import os
import sys

sys.path.insert(0, "/opt/trn_rl_repo")

import numpy as np

B = 16384
NSTEPS = 100
DIMW = 3
DT = 0.01
SQRT_DT = DT**0.5
SIGMA0 = 0.5
NCORES = 8
BC = B // NCORES  # 2048 paths per core
NCH = BC // 128  # 16 chunks of 128 paths
NQ = 4  # quarters of the noise for DMA staging
NG = 2  # software-pipelined path groups
GCH = NCH // NG  # 8 chunks per group

LAST_EXEC_NS = None
LAST_RESULTS = None

_CACHE = {}


def _build(nsteps, debug=False):
    import concourse.tile as tile
    from concourse import bacc, mybir

    f32 = mybir.dt.float32
    bf16 = mybir.dt.bfloat16
    AF = mybir.ActivationFunctionType
    ALU = mybir.AluOpType
    AX = mybir.AxisListType

    nc = bacc.Bacc("TRN2", target_bir_lowering=False, debug=False, num_devices=NCORES)

    # ---------------- DRAM I/O ----------------
    QSTEPS = (nsteps + NQ - 1) // NQ
    dWf_d = [
        nc.dram_tensor(f"dWf{q}", [128, QSTEPS * 48], f32, kind="ExternalInput").ap()
        for q in range(NQ)
    ]
    dZf_d = [
        nc.dram_tensor(f"dZf{q}", [128, QSTEPS * 48], f32, kind="ExternalInput").ap()
        for q in range(NQ)
    ]
    L1b_d = nc.dram_tensor("L1b", [128, NCH * 128], f32, kind="ExternalInput").ap()
    W1c_d = nc.dram_tensor("W1c", [2, 128], f32, kind="ExternalInput").ap()
    W2bd_d = nc.dram_tensor("W2bd", [128, 128], f32, kind="ExternalInput").ap()
    W3c_d = nc.dram_tensor("W3c", [128, 4], f32, kind="ExternalInput").ap()
    b1c_d = nc.dram_tensor("b1c", [128, 1], f32, kind="ExternalInput").ap()
    b2c_d = nc.dram_tensor("b2c", [128, 1], f32, kind="ExternalInput").ap()
    b3c_d = nc.dram_tensor("b3c", [1, 4], f32, kind="ExternalInput").ap()
    tvals_d = nc.dram_tensor("tvals", [1, nsteps], f32, kind="ExternalInput").ap()
    ones_row_d = nc.dram_tensor("ones_row", [1, 128], f32, kind="ExternalInput").ap()
    I128_d = nc.dram_tensor("I128", [128, 128], f32, kind="ExternalInput").ap()
    y_row_d = nc.dram_tensor("y_row", [16, 128], f32, kind="ExternalInput").ap()
    y_fold_d = nc.dram_tensor("y_fold", [128, 16], f32, kind="ExternalInput").ap()
    Y_init_d = nc.dram_tensor("Y_init", [128, 16], f32, kind="ExternalInput").ap()

    loss_out = nc.dram_tensor("loss_out", [1, 1], f32, kind="ExternalOutput").ap()
    if debug:
        y_out = nc.dram_tensor("y_out", [16, 128], f32, kind="ExternalOutput").ap()
        Y_out = nc.dram_tensor("Y_out", [128, 16], f32, kind="ExternalOutput").ap()

    with tile.TileContext(nc) as tc:
        from contextlib import ExitStack

        with ExitStack() as ctx:
            cpool = ctx.enter_context(tc.tile_pool(name="const", bufs=1))
            h1pool = ctx.enter_context(tc.tile_pool(name="h1sb", bufs=3))
            h2pool = ctx.enter_context(tc.tile_pool(name="h2sb", bufs=3))
            epool = ctx.enter_context(tc.tile_pool(name="epil", bufs=2))
            pmm = ctx.enter_context(tc.tile_pool(name="pmm", bufs=5, space="PSUM"))
            pzq = ctx.enter_context(tc.tile_pool(name="pzq", bufs=2, space="PSUM"))
            pyy = ctx.enter_context(tc.tile_pool(name="pyy", bufs=1, space="PSUM"))

            # ------------- persistent SBUF tiles -------------
            dWs = [cpool.tile([128, QSTEPS * 48], f32, tag=f"dw{q}", name=f"dws{q}") for q in range(NQ)]
            dZs = [cpool.tile([128, QSTEPS * 48], f32, tag=f"dz{q}", name=f"dzs{q}") for q in range(NQ)]
            swp = cpool.tile([128, nsteps * 16], f32, tag="swp")
            L1b_bf = cpool.tile([128, NCH * 128], bf16, tag="l1b")
            W2bd_bf = cpool.tile([128, 128], bf16, tag="w2bd")
            W3_bf = cpool.tile([128, 4], bf16, tag="w3")
            W3_f = cpool.tile([128, 4], f32, tag="w3f")
            b1tab = cpool.tile([128, nsteps], f32, tag="b1tab")
            b1c_sb = cpool.tile([128, 1], f32, tag="b1c")
            b2c_sb = cpool.tile([128, 1], f32, tag="b2c")
            b3s = cpool.tile([1, 4], f32, tag="b3s")
            b3f = cpool.tile([1, 4], f32, tag="b3f")
            b3rep = cpool.tile([1, 64], bf16, tag="b3rep")
            ones_bf = cpool.tile([1, 128], bf16, tag="ones_bf")
            I128 = cpool.tile([128, 128], f32, tag="i128")
            lossT16 = cpool.tile([128, 16], f32, tag="lossT16")
            W1c_sb = cpool.tile([2, 128], f32, tag="w1c")
            tvals = cpool.tile([1, nsteps], f32, tag="tvals")
            ybf = cpool.tile([128, 128], bf16, tag="ybf")
            y_fold = cpool.tile([128, 16], f32, tag="y_fold")
            Y_f = cpool.tile([128, 16], f32, tag="Yf")
            ysq16 = cpool.tile([16, 128], f32, tag="ysq16")
            ee = cpool.tile([128, 16], f32, tag="ee")
            loss1 = cpool.tile([1, 1], f32, tag="loss1")
            lcol = cpool.tile([128, 1], f32, tag="lcol")

            # persistent PSUM y state, [16, 128] f32
            yps = pyy.tile([16, 128], f32, tag="y0")

            # ------------- init: DMAs -------------
            for q in range(NQ):
                nc.sync.dma_start(dWs[q][:], dWf_d[q][:])
                nc.sync.dma_start(dZs[q][:], dZf_d[q][:])
            nc.gpsimd.dma_start(L1b_bf[:], L1b_d[:])
            nc.vector.memset(ybf[:], 0.0)
            nc.gpsimd.dma_start(ybf[0:16, :], y_row_d[:])
            nc.gpsimd.dma_start(W2bd_bf[:], W2bd_d[:])
            nc.gpsimd.dma_start(ones_bf[:], ones_row_d[:])
            nc.sync.dma_start(W3_f[:], W3c_d[:])
            nc.sync.dma_start(b1c_sb[:], b1c_d[:])
            nc.sync.dma_start(b2c_sb[:], b2c_d[:])
            nc.sync.dma_start(b3f[:], b3c_d[:])
            nc.sync.dma_start(I128[:], I128_d[:])
            nc.sync.dma_start(W1c_sb[:], W1c_d[:])
            nc.sync.dma_start(tvals[:], tvals_d[:])
            nc.sync.dma_start(y_fold[:], y_fold_d[:])
            nc.sync.dma_start(Y_f[:], Y_init_d[:])

            # ------------- init: compute -------------
            # b1tab[:, i] = b1c + t_i * W1[0, :]   (fp32 matmul, exact)
            ps = pmm.tile([128, 512], f32, tag="mm")
            nc.tensor.matmul(
                ps[:, 0:nsteps], W1c_sb[0:1, :], tvals[0:1, :], start=True, stop=True
            )
            nc.scalar.activation(
                b1tab[:], ps[:, 0:nsteps], AF.Identity, bias=b1c_sb[:, 0:1]
            )

            # W3 scaling: z-cols * sqrt(dt), q-col * dt  (cast to bf16)
            nc.vector.tensor_scalar_mul(W3_bf[:, 0:3], W3_f[:, 0:3], float(SQRT_DT))
            nc.vector.tensor_scalar_mul(W3_bf[:, 3:4], W3_f[:, 3:4], float(DT))
            # b3 scaling + replicate x16 into bf16 row
            nc.vector.tensor_scalar_mul(b3s[0:1, 0:3], b3f[0:1, 0:3], float(SQRT_DT))
            nc.vector.tensor_scalar_mul(b3s[0:1, 3:4], b3f[0:1, 3:4], float(DT))
            nc.vector.tensor_copy(b3rep[0:1, 0:4], b3s[0:1, :])
            nc.vector.tensor_copy(b3rep[0:1, 4:8], b3rep[0:1, 0:4])
            nc.vector.tensor_copy(b3rep[0:1, 8:16], b3rep[0:1, 0:8])
            nc.vector.tensor_copy(b3rep[0:1, 16:32], b3rep[0:1, 0:16])
            nc.vector.tensor_copy(b3rep[0:1, 32:64], b3rep[0:1, 0:32])

            nc.gpsimd.memset(lossT16[:], 0.0)

            # sw prepass: swp[:, i*16+c] = sigma0*sqrt(dt) * sum_j dW[i,c*128+p,j]
            for q in range(NQ):
                nsq = max(0, min(nsteps, (q + 1) * QSTEPS) - q * QSTEPS)
                if nsq == 0:
                    continue
                lo = q * QSTEPS * 16
                src = dWs[q][:, 0 : nsq * 48].rearrange("p (s j) -> p s j", j=3)
                nc.vector.tensor_reduce(
                    swp[:, lo : lo + nsq * 16], src, axis=AX.X, op=ALU.add
                )
            nc.vector.tensor_scalar_mul(swp[:], swp[:], float(SIGMA0 * SQRT_DT))
            # fold y0 into the step-0 drift so the first transpose-accumulate
            # (start=True) seeds y1 = y0 + dt*q0 + sw0
            nc.vector.tensor_tensor(
                swp[:, 0:16], swp[:, 0:16], y_fold[:], op=ALU.add
            )

            def relu_chunk(eng, dst, src, bias_ap):
                if eng == "S":
                    nc.scalar.activation(dst, src, AF.Relu, bias=bias_ap)
                else:
                    nc.vector.tensor_scalar(
                        dst, src, bias_ap, 0.0, op0=ALU.add, op1=ALU.max
                    )

            SC_F = float((0.5 / DT) ** 0.5)  # fDT = (SC_F * qDT)^2 = 0.5*dt*q^2
            for i in range(nsteps):
                qi, ri = divmod(i, QSTEPS)
                dwf_i = dWs[qi][:, ri * 48 : (ri + 1) * 48].rearrange(
                    "p (c j) -> p c j", j=3
                )
                dzf_i = dZs[qi][:, ri * 48 : (ri + 1) * 48].rearrange(
                    "p (c j) -> p c j", j=3
                )
                zz = epool.tile([128, 96], f32, tag="zz", name=f"zze{i}")
                uv = epool.tile([128, 32], f32, tag="uv", name=f"uve{i}")
                uvt = epool.tile([128, 32], f32, tag="uvt", name=f"uvt{i}")
                r_t = epool.tile([128, 16], f32, tag="r", name=f"re{i}")
                rr_t = epool.tile([128, 16], f32, tag="rr", name=f"rre{i}")
                fDT = epool.tile([128, 16], f32, tag="fdt", name=f"fdte{i}")
                qsc = epool.tile([128, 16], f32, tag="qsc", name=f"qsc{i}")
                umf = epool.tile([128, 16], f32, tag="umf", name=f"umfe{i}")
                zqf_sb = epool.tile([128, 64], f32, tag="zqsb", name=f"zqsb{i}")
                incr = epool.tile([128, 16], f32, tag="incr", name=f"incr{i}")

                # relu chunk engine map: 4 S / 4 V, with one V chunk moved to
                # S on even steps (S is the faster funnel engine)
                r1eng = [["S", "V"], ["S", "V"]]  # [group][chunk]
                r2eng = [["S", "V"], ["S", "V" if i % 2 else "S"]]

                # --- L1 for both groups (PE stays busy back-to-back)
                h1ps = [
                    [pmm.tile([128, 512], f32, tag="mm", name=f"h1ps{i}_{g}_{k}") for k in range(2)]
                    for g in range(NG)
                ]
                for g in range(NG):
                    for cc in range(GCH):
                        c = g * GCH + cc
                        s, o = divmod(cc, 4)
                        nc.tensor.matmul(
                            h1ps[g][s][:, o * 128 : (o + 1) * 128],
                            L1b_bf[:, c * 128 : (c + 1) * 128],
                            ybf[:],
                            start=True,
                            stop=True,
                        )

                # --- relu1
                h1sb = [h1pool.tile([128, 1024], bf16, tag=f"h1_{g}", name=f"h1sb{i}_{g}") for g in range(NG)]
                for g in range(NG):
                    for s in range(2):
                        relu_chunk(
                            r1eng[g][s],
                            h1sb[g][:, s * 512 : (s + 1) * 512],
                            h1ps[g][s][:],
                            b1tab[:, i : i + 1],
                        )

                # --- L2
                h2ps = [
                    [pmm.tile([128, 512], f32, tag="mm", name=f"h2ps{i}_{g}_{k}") for k in range(2)]
                    for g in range(NG)
                ]
                for g in range(NG):
                    for s in range(2):
                        nc.tensor.matmul(
                            h2ps[g][s][:],
                            W2bd_bf[:],
                            h1sb[g][:, s * 512 : (s + 1) * 512],
                            start=True,
                            stop=True,
                        )

                # --- relu2
                h2sb = [h2pool.tile([128, 1024], bf16, tag=f"h2_{g}", name=f"h2sb{i}_{g}") for g in range(NG)]
                for g in range(NG):
                    for s in range(2):
                        relu_chunk(
                            r2eng[g][s],
                            h2sb[g][:, s * 512 : (s + 1) * 512],
                            h2ps[g][s][:],
                            b2c_sb[:, 0:1],
                        )

                # --- L3 both groups into one PSUM bank
                zqf_ps = pzq.tile([128, 64], f32, tag="zq", name=f"zq{i}")
                nc.tensor.matmul(
                    zqf_ps[:], ones_bf[0:1, :], b3rep[0:1, :], start=True, stop=False
                )
                for g in range(NG):
                    for cc in range(GCH):
                        c = g * GCH + cc
                        nc.tensor.matmul(
                            zqf_ps[:, c * 4 : (c + 1) * 4],
                            h2sb[g][:, cc * 128 : (cc + 1) * 128],
                            W3_bf[:],
                            start=False,
                            stop=(c == NCH - 1),
                            skip_group_check=True,
                        )

                # --- y chain: incr (V) -> transpose-accumulate into
                # persistent PSUM y state -> bf16 copy for next L1 (V)
                qv_ps = zqf_ps[:].rearrange("p (c m) -> p c m", m=4)[:, :, 3:4]
                nc.vector.tensor_tensor(
                    incr[:].rearrange("p (c o) -> p c o", o=1),
                    qv_ps,
                    swp[:, i * 16 : (i + 1) * 16].rearrange("p (c o) -> p c o", o=1),
                    op=ALU.add,
                )
                nc.tensor.matmul(
                    yps[:],
                    incr[:],
                    I128[:],
                    is_transpose=True,
                    start=(i == 0),
                    stop=(i == nsteps - 1),
                    skip_group_check=True,
                )
                nc.vector.tensor_copy(ybf[0:16, :], yps[:])

                # --- epilogue off-chain: zq to SBUF (S), math on GpSimd
                nc.scalar.activation(zqf_sb[:], zqf_ps[:], AF.Copy)
                zview = zqf_sb[:].rearrange("p (c m) -> p c m", m=4)[:, :, 0:3]
                qview = zqf_sb[:].rearrange("p (c m) -> p c m", m=4)[:, :, 3:4]
                zz0 = zz[:, 0:48].rearrange("p (c j) -> p c j", j=3)
                zz1 = zz[:, 48:96].rearrange("p (c j) -> p c j", j=3)
                nc.gpsimd.tensor_tensor(zz0, zview, dwf_i, op=ALU.mult)
                nc.gpsimd.tensor_tensor(zz1, zview, dzf_i, op=ALU.mult)
                zzv = zz[:].rearrange("p (h j) -> p h j", j=3)
                nc.gpsimd.tensor_tensor(
                    uvt[:], zzv[:, :, 0], zzv[:, :, 1], op=ALU.add
                )
                nc.gpsimd.tensor_tensor(uv[:], uvt[:], zzv[:, :, 2], op=ALU.add)
                nc.gpsimd.tensor_tensor(
                    r_t[:], uv[:, 0:16], uv[:, 16:32], op=ALU.subtract
                )
                nc.gpsimd.tensor_tensor(rr_t[:], r_t[:], r_t[:], op=ALU.mult)
                nc.gpsimd.tensor_tensor(lossT16[:], lossT16[:], rr_t[:], op=ALU.add)
                nc.gpsimd.tensor_scalar_mul(qsc[:], qview, SC_F)
                nc.gpsimd.tensor_tensor(fDT[:], qsc[:], qsc[:], op=ALU.mult)
                nc.gpsimd.tensor_tensor(umf[:], uv[:, 0:16], fDT[:], op=ALU.subtract)
                nc.gpsimd.tensor_tensor(Y_f[:], Y_f[:], umf[:], op=ALU.add)

            # ------------- terminal loss -------------
            nc.scalar.activation(ysq16[:], yps[:], AF.Square)
            ysq_ps = pzq.tile([128, 16], f32, tag="zq", name="ysqps")
            nc.tensor.matmul(ysq_ps[:], ysq16[:], I128[0:16, 0:16], is_transpose=True)
            nc.vector.tensor_tensor(ee[:], Y_f[:], ysq_ps[:], op=ALU.subtract)
            nc.scalar.activation(ee[:], ee[:], AF.Square)
            nc.vector.tensor_tensor(lossT16[:], lossT16[:], ee[:], op=ALU.add)
            nc.vector.tensor_reduce(
                lcol[:],
                lossT16[:].rearrange("p (o c) -> p o c", o=1),
                axis=AX.X,
                op=ALU.add,
            )
            lrow_ps = pzq.tile([1, 128], f32, tag="zq", name="lrowps")
            nc.tensor.matmul(lrow_ps[0:1, 0:128], lcol[:, 0:1], I128[:], is_transpose=True)
            nc.vector.tensor_reduce(
                loss1[:],
                lrow_ps[0:1, 0:128].rearrange("p (o c) -> p o c", o=1),
                axis=AX.X,
                op=ALU.add,
            )
            nc.vector.tensor_scalar_mul(loss1[:], loss1[:], 1.0 / B)
            nc.sync.dma_start(loss_out[:], loss1[:])
            if debug:
                nc.sync.dma_start(y_out[:], ysq16[:])
                nc.sync.dma_start(Y_out[:], Y_f[:])

    nc.compile()
    return nc


def _host_inputs(nsteps, y0, Y0, zW1, zb1, zW2, zb2, zW3, zb3, qW1, qb1, qW2, qb2, qW3, qb3, dW, dZ):
    """Per-core input maps. Layout/slicing only — no arithmetic on inputs."""
    f = np.float32
    QSTEPS = (nsteps + NQ - 1) // NQ
    W1row1 = np.concatenate([zW1[1], qW1[1]]).astype(f)  # (128,)
    L1b = np.zeros((128, NCH * 128), f)
    for c in range(NCH):
        L1b[c, c * 128 : (c + 1) * 128] = W1row1
    W1c = np.concatenate([zW1, qW1], axis=1).astype(f)  # (2,128)
    W2bd = np.zeros((128, 128), f)
    W2bd[0:64, 0:64] = zW2
    W2bd[64:128, 64:128] = qW2
    W3c = np.zeros((128, 4), f)
    W3c[0:64, 0:3] = zW3
    W3c[64:128, 3] = qW3[:, 0]
    b1c = np.concatenate([zb1, qb1]).astype(f).reshape(128, 1)
    b2c = np.concatenate([zb2, qb2]).astype(f).reshape(128, 1)
    b3c = np.concatenate([zb3, qb3]).astype(f).reshape(1, 4)
    tvals = (np.arange(nsteps) * DT).astype(f).reshape(1, nsteps)
    ones_row = np.ones((1, 128), f)
    I128 = np.eye(128, dtype=f)
    y_row = np.broadcast_to(np.asarray(y0, f).reshape(1, 1), (16, 128)).copy()
    y_fold = np.broadcast_to(np.asarray(y0, f).reshape(1, 1), (128, 16)).copy()
    Y_init = np.broadcast_to(np.asarray(Y0, f).reshape(1, 1), (128, 16)).copy()

    shared = dict(
        W1c=W1c, W2bd=W2bd, W3c=W3c, b1c=b1c, b2c=b2c, b3c=b3c,
        tvals=tvals, ones_row=ones_row, I128=I128,
        y_row=y_row, y_fold=y_fold, Y_init=Y_init,
    )
    shared["L1b"] = L1b

    in_maps = []
    for core in range(NCORES):
        o = core * BC
        m = dict(shared)
        for name, arr in (("dWf", dW), ("dZf", dZ)):
            # fold: [nsteps, 2048, 3] -> [128, nsteps*48],
            # col = i*48 + c*3 + j, path = c*128 + p
            x = np.ascontiguousarray(arr[:nsteps, o : o + BC, :]).astype(f)
            x = x.reshape(nsteps, NCH, 128, 3).transpose(2, 0, 1, 3)
            x = np.ascontiguousarray(x).reshape(128, nsteps * 48)
            for q in range(NQ):
                sl = x[:, q * QSTEPS * 48 : (q + 1) * QSTEPS * 48]
                buf = np.zeros((128, QSTEPS * 48), f)
                buf[:, : sl.shape[1]] = sl
                m[f"{name}{q}"] = buf
        in_maps.append(m)
    return in_maps


def _run(nsteps, inputs, debug=False):
    global LAST_EXEC_NS, LAST_RESULTS
    from concourse import bass_utils

    key = (nsteps, debug)
    if key not in _CACHE:
        _CACHE[key] = _build(nsteps, debug=debug)
    nc = _CACHE[key]

    in_maps = _host_inputs(nsteps, **inputs)
    trace = bool(os.environ.get("BASS_TRACE"))
    kwargs = {}
    if trace:
        import tempfile

        kwargs = dict(trace=True, tmpdir=tempfile.mkdtemp(prefix="bsde_trace_"))
    res = bass_utils.run_bass_kernel_spmd(
        nc, in_maps, core_ids=list(range(NCORES)), **kwargs
    )
    LAST_RESULTS = res
    LAST_EXEC_NS = res.exec_time_ns
    return res


def kernel(**inputs):
    inputs = {k: np.asarray(v, np.float32) for k, v in inputs.items()}
    res = _run(NSTEPS, inputs, debug=False)
    total = np.float32(0.0)
    for core in range(NCORES):
        total += res.results[core]["loss_out"][0, 0]
    return np.array(total, dtype=np.float32)
